# revision 19
# baseline (speedup 1.0000x reference)
"""Trainium2 Bass kernel for LrmcSeededPoolGCN (8 NeuronCores, SPMD).

Strategy: nodes are relabeled (sorted by cluster_id) and partitioned into 8
contiguous shards of 6250 (padded 6272) nodes; edges are sharded by
destination. Each core:
  1. computes its shard of h' = dinv * (x @ W1)  (bf16, padded to 128 cols)
     and AllGathers the full table to HBM,
  2. gathers h'[src] for its (dst-sorted, window-padded) edges with
     dma_gather, reduces segments with one-hot matmuls on the TensorEngine
     (PSUM accumulation per 128-dst window); the per-edge normalization
     dinv[src]*dinv[dst] is folded into the table and the window eviction,
  3. pools x1 by cluster with the same one-hot matmul trick (ones column
     appended -> sums and counts in one shot), AllReduces [65,1024],
  4. runs the small pooled-graph GCN replicated (A_hat shipped as a dense
     0/1 matrix built host-side from the edge cluster pairs), writes
     x_p2 (+b2, +relu(alpha)*b_skip folded) k-major to HBM,
  5. gathers x_p2[cluster] per node, adds the skip matmul, writes logits.

All floating point math runs on device; the host only does integer index
preparation (permutation, edge bucketing/padding, one-hot relative indices)
and the final unshard (inverse permutation).
"""

import os
import sys

import numpy as np

if "/opt/trn_rl_repo" not in sys.path:
    sys.path.insert(0, "/opt/trn_rl_repo")

N, E, K, INF, H, OUTF = 50000, 800000, 1024, 128, 64, 64
NCORES = 8
SH = N // NCORES            # 6250 nodes per core
WPC = 49                    # 128-node windows per core
SHP = WPC * 128             # 6272 padded nodes per core
LOCUT = 32768               # int16 index split point for the gather table
TABROWS = NCORES * SHP      # 50176 rows in the all-gathered h' table
NCHUNK = 7                  # gather chunks per core (7 windows each)
WPCH = WPC // NCHUNK        # 7 windows per chunk


# --------------------------------------------------------------------------
# host-side index preparation (integer-only)
# --------------------------------------------------------------------------
def _host_prep(x, edge_index, cluster_id):
    order = np.argsort(cluster_id, kind="stable").astype(np.int64)
    inv = np.empty(N, np.int64)
    inv[order] = np.arange(N)
    clusterP = np.asarray(cluster_id, np.int64)[order]

    src = np.asarray(edge_index[0], np.int64)
    dst = np.asarray(edge_index[1], np.int64)
    deg = (np.bincount(dst, minlength=N) + 1).astype(np.float32)
    degP = deg[order]

    s_new = inv[src]
    d_new = inv[dst]
    core_of = d_new // SH
    dpos = d_new % SH
    w_of = dpos // 128
    drel_of = (dpos % 128).astype(np.float32)
    # gather-table row of the source node (shard-padded, partition-major):
    # node v in shard c at local pos l -> row c*SHP + (l%128)*WPC + l//128
    sc, sl = s_new // SH, s_new % SH
    r = sc * SHP + (sl % 128) * WPC + sl // 128
    half = (r >= LOCUT).astype(np.int64)  # 0 = lo, 1 = hi

    grp = (core_of * WPC + w_of) * 2 + half
    ngrp = NCORES * WPC * 2
    cnt = np.bincount(grp, minlength=ngrp)
    cnt_lo = cnt[0::2].reshape(NCORES, WPC)
    cnt_hi = cnt[1::2].reshape(NCORES, WPC)
    BL = max(1, int(-(-cnt_lo.max() // 128)))
    BH = max(1, int(-(-cnt_hi.max() // 128)))

    eorder = np.argsort(grp, kind="stable")
    grp_s = grp[eorder]
    starts = np.zeros(ngrp, np.int64)
    np.cumsum(cnt[:-1], out=starts[1:])
    within = np.arange(E, dtype=np.int64) - starts[grp_s]

    gidx_lo = np.zeros((NCORES, WPC * BL * 128), np.int16)
    gidx_hi = np.zeros((NCORES, WPC * BH * 128), np.int16)
    drel_lo = np.full((NCORES, WPC * BL * 128), -1.0, np.float32)
    drel_hi = np.full((NCORES, WPC * BH * 128), -1.0, np.float32)

    c_s, w_s, h_s = core_of[eorder], w_of[eorder], half[eorder]
    r_s, dr_s = r[eorder], drel_of[eorder]
    m = h_s == 0
    slot = w_s[m] * (BL * 128) + within[m]
    gidx_lo[c_s[m], slot] = r_s[m].astype(np.int16)
    drel_lo[c_s[m], slot] = dr_s[m]
    m = h_s == 1
    slot = w_s[m] * (BH * 128) + within[m]
    gidx_hi[c_s[m], slot] = (r_s[m] - LOCUT).astype(np.int16)
    drel_hi[c_s[m], slot] = dr_s[m]

    # wrap gather indices: flat slot s -> [s%16, s//16]
    def wrap16(a):
        w = a.reshape(NCORES, -1, 16).transpose(0, 2, 1)  # [NC, 16, n/16]
        return np.ascontiguousarray(np.tile(w, (1, 8, 1)))  # [NC, 128, n/16]

    # one-hot relative values: flat slot s -> [partition s%128, block s//128]
    def bywin(a):
        return np.ascontiguousarray(
            a.reshape(NCORES, -1, 128).transpose(0, 2, 1))

    # cluster one-hot relatives per node block, for the 8 global windows
    clusP = np.full((NCORES, SHP), -1, np.int64)
    for c in range(NCORES):
        clusP[c, :SH] = clusterP[c * SH:(c + 1) * SH]
    g = np.arange(8, dtype=np.int64)
    rel = clusP[:, :, None] - (g * 128)[None, None, :]  # [NC, SHP, 8]
    rel = np.where((rel >= 0) & (rel < 128), rel, -1).astype(np.float32)
    # -> [NC, 128, WPC*8]: node v=w*128+p, col w*8+g
    cl_rel = np.ascontiguousarray(
        rel.reshape(NCORES, WPC, 128, 8)
        .transpose(0, 2, 1, 3).reshape(NCORES, 128, WPC * 8))

    # x_p2 gather indices per node: row of xp2_dram == cluster id
    r2 = np.where(clusP >= 0, clusP, 0)
    upidx = wrap16(r2.astype(np.int16))  # [NC, 128, SHP//16]

    # per-core xT (feature-major, padded) and deg
    xT = np.zeros((NCORES, INF, SHP), np.float32)
    degsh = np.ones((NCORES, 128, WPC), np.float32)
    xp = np.asarray(x, np.float32)[order]
    for c in range(NCORES):
        xT[c, :, :SH] = xp[c * SH:(c + 1) * SH].T
        degsh[c] = np.pad(degP[c * SH:(c + 1) * SH],
                          (0, SHP - SH), constant_values=1.0
                          ).reshape(WPC, 128).T

    # pooled adjacency A_hat (dense 0/1 + I), rearranged [128, 8*K]
    cu = np.asarray(cluster_id, np.int64)[src]
    cv = np.asarray(cluster_id, np.int64)[dst]
    A = np.zeros((K, K), np.float32)
    A[cu, cv] = 1.0
    np.fill_diagonal(A, 1.0)
    ahat = np.ascontiguousarray(
        A.reshape(8, 128, K).transpose(1, 0, 2).reshape(128, 8 * K))

    meta = dict(order=order, BL=BL, BH=BH)
    arrays = dict(xT=xT, degsh=degsh, gidx_lo=wrap16(gidx_lo),
                  gidx_hi=wrap16(gidx_hi), drel_lo=bywin(drel_lo),
                  drel_hi=bywin(drel_hi), cl_rel=cl_rel, upidx=upidx,
                  ahat=ahat)
    return meta, arrays


# --------------------------------------------------------------------------
# device program
# --------------------------------------------------------------------------
def _build(BL, BH, add_b1, add_b2, add_bsk):
    import concourse.bacc as bacc
    import concourse.bass as bass
    import concourse.mybir as mybir
    import concourse.tile as tile

    f32 = mybir.dt.float32
    bf16 = mybir.dt.bfloat16
    i16 = mybir.dt.int16
    AF = mybir.ActivationFunctionType
    ALU = mybir.AluOpType

    nc = bacc.Bacc("TRN2", target_bir_lowering=False, debug=False,
                   num_devices=NCORES)

    # ---- parameters (per-core arrays arrive via in_maps) ----
    p_xT = nc.declare_dram_parameter("xT", [INF, SHP], f32, isOutput=False)
    p_deg = nc.declare_dram_parameter("degsh", [128, WPC], f32, isOutput=False)
    p_W1 = nc.declare_dram_parameter("W1", [INF, H], f32, isOutput=False)
    p_W2 = nc.declare_dram_parameter("W2", [H, OUTF], f32, isOutput=False)
    p_Wsk = nc.declare_dram_parameter("Wsk", [H, OUTF], f32, isOutput=False)
    p_b1 = nc.declare_dram_parameter("b1r", [1, H], f32, isOutput=False)
    p_b2 = nc.declare_dram_parameter("b2c", [OUTF, 1], f32, isOutput=False)
    p_bsk = nc.declare_dram_parameter("bskr", [1, OUTF], f32, isOutput=False)
    p_alpha = nc.declare_dram_parameter("alpha11", [1, 1], f32, isOutput=False)
    p_iota = nc.declare_dram_parameter("iota128", [128, 128], f32, isOutput=False)
    p_idb = nc.declare_dram_parameter("ident", [128, 128], f32, isOutput=False)
    p_glo = nc.declare_dram_parameter("gidx_lo", [128, WPC * BL * 8], i16, isOutput=False)
    p_ghi = nc.declare_dram_parameter("gidx_hi", [128, WPC * BH * 8], i16, isOutput=False)
    p_dlo = nc.declare_dram_parameter("drel_lo", [128, WPC * BL], f32, isOutput=False)
    p_dhi = nc.declare_dram_parameter("drel_hi", [128, WPC * BH], f32, isOutput=False)
    p_crel = nc.declare_dram_parameter("cl_rel", [128, WPC * 8], f32, isOutput=False)
    p_upidx = nc.declare_dram_parameter("upidx", [128, SHP // 16], i16, isOutput=False)
    p_ahat = nc.declare_dram_parameter("ahat", [128, 8 * K], f32, isOutput=False)
    p_out = nc.declare_dram_parameter("out", [128, WPC * OUTF], f32, isOutput=True)

    # ---- internal DRAM ----
    dbg = os.environ.get("BASSGNN_DEBUG") == "1"
    if dbg:
        dbg_dinvp = nc.dram_tensor("dbg_dinvp", [1, K], f32)
        dbg_xpT = nc.dram_tensor("dbg_xpT", [H, K], mybir.dt.bfloat16)
        dbg_hps = nc.dram_tensor("dbg_hps", [128, 8 * H], mybir.dt.bfloat16)
        dbg_xp2T = nc.dram_tensor("dbg_xp2T", [H, K], f32)
        dbg_up = nc.dram_tensor("dbg_up", [128, WPC * OUTF], f32)
    hsh_dram = nc.dram_tensor("hsh", [128, SHP], bf16)
    htab = nc.dram_tensor("htab", [TABROWS, 128], bf16, addr_space="Shared")
    arin = nc.dram_tensor("arin", [H + 1, K], f32)
    arout = nc.dram_tensor("arout", [H + 1, K], f32, addr_space="Shared")
    xp2_dram = nc.dram_tensor("xp2d", [K, OUTF], f32)

    cores = [list(range(NCORES))]

    with tile.TileContext(nc) as tc:
        with tc.tile_pool(name="sb", bufs=1) as sb, \
             tc.tile_pool(name="ps", bufs=1, space="PSUM") as ps:

            # ---------------- constants ----------------
            W1b = sb.tile([INF, H], bf16)
            nc.gpsimd.dma_start(W1b[:], p_W1[:])
            W2b = sb.tile([H, OUTF], bf16)
            nc.gpsimd.dma_start(W2b[:], p_W2[:])
            Wskf = sb.tile([H, OUTF], f32)
            nc.sync.dma_start(Wskf[:], p_Wsk[:])
            iota_sb = sb.tile([128, 128], bf16)
            nc.gpsimd.dma_start(iota_sb[:], p_iota[:])
            identb = sb.tile([128, 128], bf16)
            nc.gpsimd.dma_start(identb[:], p_idb[:])
            identf = sb.tile([128, 128], f32)
            nc.sync.dma_start(identf[:], p_idb[:])
            b1row = sb.tile([1, H], f32)
            nc.sync.dma_start(b1row[:], p_b1[:])
            b2col = sb.tile([OUTF, 1], f32)
            nc.sync.dma_start(b2col[:], p_b2[:])
            bskrow = sb.tile([1, OUTF], f32)
            nc.sync.dma_start(bskrow[:], p_bsk[:])
            alpha_sb = sb.tile([1, 1], f32)
            nc.sync.dma_start(alpha_sb[:], p_alpha[:])
            deg_sb = sb.tile([128, WPC], f32)
            nc.sync.dma_start(deg_sb[:], p_deg[:])
            dlo_sb = sb.tile([128, WPC * BL], bf16)
            nc.gpsimd.dma_start(dlo_sb[:], p_dlo[:])
            dhi_sb = sb.tile([128, WPC * BH], bf16)
            nc.gpsimd.dma_start(dhi_sb[:], p_dhi[:])
            crel_sb = sb.tile([128, WPC * 8], bf16)
            nc.gpsimd.dma_start(crel_sb[:], p_crel[:])
            glo_sb = sb.tile([128, WPC * BL * 8], i16)
            nc.sync.dma_start(glo_sb[:], p_glo[:])
            ghi_sb = sb.tile([128, WPC * BH * 8], i16)
            nc.sync.dma_start(ghi_sb[:], p_ghi[:])
            upidx_sb = sb.tile([128, SHP // 16], i16)
            nc.sync.dma_start(upidx_sb[:], p_upidx[:])

            ones1x128 = sb.tile([1, 128], f32)
            nc.vector.memset(ones1x128[:], 1.0)
            onescol_b = sb.tile([128, 1], bf16)
            nc.vector.memset(onescol_b[:], 1.0)

            dinv_sb = sb.tile([128, WPC], f32)
            nc.vector.reciprocal(dinv_sb[:], deg_sb[:])
            nc.scalar.activation(dinv_sb[:], dinv_sb[:], AF.Sqrt)

            # replicated b1 [128, H]
            b1rep = sb.tile([128, H], f32)
            if add_b1:
                b1ps = ps.tile([128, H], f32, tag="psA", bufs=2)
                nc.tensor.matmul(b1ps[:], ones1x128[:], b1row[:])
                nc.vector.tensor_copy(b1rep[:], b1ps[:])

            # ---------------- phase 1: h' shard + AllGather ----------------
            xT_sb = sb.tile([INF, SHP], bf16, tag="tA", bufs=2)
            nc.gpsimd.dma_start(xT_sb[:], p_xT[:])
            hq_sb = sb.tile([128, SHP], bf16, tag="tB", bufs=2)
            nc.vector.memset(hq_sb[:], 0.0)
            hown = sb.tile([128, WPC * H], f32)
            for w in range(WPC):
                hps = ps.tile([128, H], f32, tag="psA", bufs=2, name=f"hps{w}")
                nc.tensor.matmul(hps[:], xT_sb[:, w * 128:(w + 1) * 128], W1b[:])
                nc.vector.tensor_scalar(
                    hq_sb[:, w * 128:w * 128 + H], hps[:],
                    dinv_sb[:, w:w + 1], None, ALU.mult)
                nc.vector.tensor_scalar(
                    hown[:, w * H:(w + 1) * H], hps[:],
                    dinv_sb[:, w:w + 1], None, ALU.mult)
            nc.sync.dma_start(hsh_dram[:], hq_sb[:])
            nc.gpsimd.collective_compute(
                "AllGather", mybir.AluOpType.bypass, replica_groups=cores,
                ins=[hsh_dram[:]], outs=[htab[:]])

            # ---------------- phase 2: edge gather + segment reduce --------
            x1e = sb.tile([128, WPC * (H + 1)], bf16)
            # ones column at col w*(H+1)+H for counting in the pooling matmul
            nc.vector.memset(
                x1e[:].rearrange("p (w f) -> p w f", f=H + 1)[:, :, H:H + 1], 1.0)

            nlo, nhi = WPCH * BL * 128, WPCH * BH * 128
            for chunk in range(NCHUNK):
                lo_t = sb.tile([128, WPCH * BL, 128], bf16, tag="tB", bufs=2,
                               name=f"lo{chunk}")
                nc.gpsimd.dma_gather(
                    lo_t[:], htab[0:LOCUT, :],
                    glo_sb[:, chunk * (nlo // 16):(chunk + 1) * (nlo // 16)],
                    num_idxs=nlo, num_idxs_reg=nlo, elem_size=128,
                    single_packet=False)
                hi_t = sb.tile([128, WPCH * BH, 128], bf16, tag="tA", bufs=2,
                               name=f"hi{chunk}")
                nc.gpsimd.dma_gather(
                    hi_t[:], htab[LOCUT:TABROWS, :],
                    ghi_sb[:, chunk * (nhi // 16):(chunk + 1) * (nhi // 16)],
                    num_idxs=nhi, num_idxs_reg=nhi, elem_size=128,
                    single_packet=False)

                for wl in range(WPCH):
                    w = chunk * WPCH + wl
                    # batched one-hot build for this window (lo + hi blocks)
                    oh_lo = sb.tile([128, BL, 128], bf16, tag="ohlo", bufs=3,
                                    name=f"ohlo{w}")
                    nc.vector.tensor_tensor(
                        oh_lo[:],
                        iota_sb[:].rearrange("p (a j) -> p a j", a=1)
                        .broadcast_to([128, BL, 128]),
                        dlo_sb[:, w * BL:(w + 1) * BL]
                        .rearrange("p (b a) -> p b a", a=1)
                        .broadcast_to([128, BL, 128]),
                        ALU.is_equal)
                    oh_hi = sb.tile([128, BH, 128], bf16, tag="ohhi", bufs=3,
                                    name=f"ohhi{w}")
                    nc.vector.tensor_tensor(
                        oh_hi[:],
                        iota_sb[:].rearrange("p (a j) -> p a j", a=1)
                        .broadcast_to([128, BH, 128]),
                        dhi_sb[:, w * BH:(w + 1) * BH]
                        .rearrange("p (b a) -> p b a", a=1)
                        .broadcast_to([128, BH, 128]),
                        ALU.is_equal)

                    eps = ps.tile([128, H], f32, tag="psA", bufs=2,
                                  name=f"eps{w}")
                    nb = BL + BH
                    for b in range(BL):
                        nc.tensor.matmul(
                            eps[:], oh_lo[:, b, :], lo_t[:, wl * BL + b, 0:H],
                            start=(b == 0), stop=(b == nb - 1))
                    for b in range(BH):
                        nc.tensor.matmul(
                            eps[:], oh_hi[:, b, :], hi_t[:, wl * BH + b, 0:H],
                            start=False, stop=(BL + b == nb - 1))

                    t1 = sb.tile([128, H], f32, tag="t1", bufs=3, name=f"t1{w}")
                    nc.vector.tensor_tensor(
                        t1[:], eps[:], hown[:, w * H:(w + 1) * H], ALU.add)
                    if add_b1:
                        nc.vector.tensor_scalar(
                            t1[:], t1[:], dinv_sb[:, w:w + 1], None, ALU.mult)
                        nc.vector.tensor_tensor(t1[:], t1[:], b1rep[:], ALU.add)
                        nc.scalar.activation(
                            x1e[:, w * (H + 1):w * (H + 1) + H], t1[:], AF.Relu)
                    else:
                        # x1 = relu(dinv * (segsum + h'own)) fused in ACT
                        nc.scalar.activation(
                            x1e[:, w * (H + 1):w * (H + 1) + H], t1[:], AF.Relu,
                            scale=dinv_sb[:, w:w + 1])

            # ---------------- phase 3: pooling + AllReduce -----------------
            poolps = [ps.tile([H + 1, 512], f32, tag=f"psP{i}", bufs=1,
                              name=f"poolps{i}") for i in range(2)]
            for w in range(WPC):
                ohc = sb.tile([128, 8, 128], bf16, tag="ohc", bufs=3,
                              name=f"ohc{w}")
                nc.vector.tensor_tensor(
                    ohc[:],
                    iota_sb[:].rearrange("p (a j) -> p a j", a=1)
                    .broadcast_to([128, 8, 128]),
                    crel_sb[:, w * 8:(w + 1) * 8]
                    .rearrange("p (b a) -> p b a", a=1)
                    .broadcast_to([128, 8, 128]),
                    ALU.is_equal)
                for hf in range(2):
                    nc.tensor.matmul(
                        poolps[hf][:],
                        x1e[:, w * (H + 1):(w + 1) * (H + 1)],
                        ohc[:, 4 * hf:4 * (hf + 1), :],
                        start=(w == 0), stop=(w == WPC - 1))
            arin_sb = sb.tile([H + 1, K], f32)
            nc.vector.tensor_copy(arin_sb[:, 0:512], poolps[0][:])
            nc.vector.tensor_copy(arin_sb[:, 512:1024], poolps[1][:])
            nc.sync.dma_start(arin[:], arin_sb[:])
            nc.gpsimd.collective_compute(
                "AllReduce", mybir.AluOpType.add, replica_groups=cores,
                ins=[arin[:]], outs=[arout[:]])
            arsum = sb.tile([H + 1, K], f32)
            nc.sync.dma_start(arsum[:], arout[:])

            # ---------------- phase 4: pooled-graph GCN (replicated) -------
            ahat_sb = sb.tile([128, 8 * K], bf16)
            nc.gpsimd.dma_start(ahat_sb[:], p_ahat[:])

            # counts -> 1/max(counts,1)
            recip = sb.tile([1, K], f32)
            nc.vector.tensor_scalar(recip[:], arsum[H:H + 1, :], 1.0, None,
                                    ALU.max)
            nc.vector.reciprocal(recip[:], recip[:])

            # deg_p = column sums of A_hat; dinv_p = rsqrt
            degps = [ps.tile([1, 512], f32, tag=f"psP{i}", bufs=1,
                             name=f"degps{i}") for i in range(2)]
            for ci in range(8):
                for hf in range(2):
                    nc.tensor.matmul(
                        degps[hf][:], onescol_b[:],
                        ahat_sb[:, ci * K + hf * 512: ci * K + (hf + 1) * 512],
                        start=(ci == 0), stop=(ci == 7))
            dinvp = sb.tile([1, K], f32)
            nc.vector.reciprocal(dinvp[:, 0:512], degps[0][:])
            nc.vector.reciprocal(dinvp[:, 512:1024], degps[1][:])
            nc.scalar.activation(dinvp[:], dinvp[:], AF.Sqrt)

            # replicate dinv_p across H partitions once (SBUF, reused twice)
            dinvrep = sb.tile([H, K], f32)
            for hf in range(2):
                cs = slice(hf * 512, (hf + 1) * 512)
                dreps = ps.tile([H, 512], f32, tag="psB", bufs=2,
                                name=f"dreps{hf}")
                nc.tensor.matmul(dreps[:], ones1x128[:, 0:H], dinvp[:, cs])
                nc.vector.tensor_copy(dinvrep[:, cs], dreps[:])

            # x_pT = (sums * recip) * dinvp  (dinv_p[s] folded into x_p)
            xpT = sb.tile([H, K], bf16)
            for hf in range(2):
                cs = slice(hf * 512, (hf + 1) * 512)
                reps = ps.tile([H, 512], f32, tag="psB", bufs=2,
                               name=f"reps{hf}")
                nc.tensor.matmul(reps[:], ones1x128[:, 0:H], recip[:, cs])
                tx = sb.tile([H, 512], f32, tag="tx", bufs=2, name=f"tx{hf}")
                nc.vector.tensor_tensor(tx[:], arsum[0:H, cs], reps[:],
                                        ALU.mult)
                nc.vector.tensor_tensor(xpT[:, cs], tx[:], dinvrep[:, cs],
                                        ALU.mult)

            # h_p = (dinv_p * x_p) @ W2, s-major bf16
            hps_sb = sb.tile([128, 8 * H], bf16)
            for kb in range(8):
                hpps = ps.tile([128, H], f32, tag="psA", bufs=2,
                               name=f"hpps{kb}")
                nc.tensor.matmul(hpps[:], xpT[:, kb * 128:(kb + 1) * 128],
                                 W2b[:])
                nc.vector.tensor_copy(hps_sb[:, kb * H:(kb + 1) * H], hpps[:])

            # x_p2^T = h_p'^T @ A_hat, then * dinv_p[t] + b2
            xp2T = [ps.tile([H, 512], f32, tag=f"psP{i}", bufs=1,
                            name=f"xp2T{i}") for i in range(2)]
            for kb in range(8):
                for hf in range(2):
                    nc.tensor.matmul(
                        xp2T[hf][:],
                        hps_sb[:, kb * H:(kb + 1) * H],
                        ahat_sb[:, kb * K + hf * 512: kb * K + (hf + 1) * 512],
                        start=(kb == 0), stop=(kb == 7))
            xp2T_sb = sb.tile([H, K], f32)
            for hf in range(2):
                cs = slice(hf * 512, (hf + 1) * 512)
                nc.vector.tensor_tensor(xp2T_sb[:, cs], xp2T[hf][:],
                                        dinvrep[:, cs], ALU.mult)
                if add_b2:
                    nc.vector.tensor_scalar(xp2T_sb[:, cs], xp2T_sb[:, cs],
                                            b2col[:], None, ALU.add)

            # skip-head constants: Wsk' = relu(alpha)*Wsk ; bsk' replicated
            alr = sb.tile([1, 1], f32)
            nc.scalar.activation(alr[:], alpha_sb[:], AF.Relu)
            alr64ps = ps.tile([H, 1], f32, tag="psB", bufs=2)
            nc.tensor.matmul(alr64ps[:], ones1x128[:, 0:H], alr[:])
            alr64 = sb.tile([H, 1], f32)
            nc.vector.tensor_copy(alr64[:], alr64ps[:])
            Wskb = sb.tile([H, OUTF], bf16)
            nc.vector.tensor_scalar(Wskb[:], Wskf[:], alr64[:], None, ALU.mult)
            bsk2 = sb.tile([1, OUTF], f32)
            bskrep = sb.tile([128, OUTF], f32)
            if add_bsk:
                nc.vector.tensor_scalar(bsk2[:], bskrow[:], alr[:], None,
                                        ALU.mult)
                bskps = ps.tile([128, OUTF], f32, tag="psA", bufs=2)
                nc.tensor.matmul(bskps[:], ones1x128[:], bsk2[:])
                nc.vector.tensor_copy(bskrep[:], bskps[:])

            # x_p2 k-major [K, OUTF] f32 -> DRAM (row k at [k%128, (k//128)*64])
            xp2km = sb.tile([128, 8 * OUTF], f32)
            for kb in range(8):
                tpps = ps.tile([128, H], f32, tag="psT", bufs=2,
                               name=f"tpps{kb}")
                nc.tensor.matmul(tpps[:], xp2T_sb[:, kb * 128:(kb + 1) * 128],
                                 identf[0:H, 0:H], is_transpose=True)
                if add_bsk:
                    nc.vector.tensor_tensor(
                        xp2km[:, kb * OUTF:(kb + 1) * OUTF], tpps[:],
                        bskrep[:], ALU.add)
                else:
                    nc.vector.tensor_copy(
                        xp2km[:, kb * OUTF:(kb + 1) * OUTF], tpps[:])
            for kb in range(8):
                nc.sync.dma_start(xp2_dram[kb * 128:(kb + 1) * 128, :],
                                  xp2km[:, kb * OUTF:(kb + 1) * OUTF])

            if dbg:
                nc.sync.dma_start(dbg_dinvp[:], dinvp[:])
                nc.sync.dma_start(dbg_xpT[:], xpT[:])
                nc.sync.dma_start(dbg_hps[:], hps_sb[:])
                nc.sync.dma_start(dbg_xp2T[:], xp2T_sb[:])

            # ---------------- phase 5: per-node head -----------------------
            up_t = sb.tile([128, WPC, OUTF], f32)
            nc.gpsimd.dma_gather(
                up_t[:], xp2_dram[:], upidx_sb[:],
                num_idxs=SHP, num_idxs_reg=SHP, elem_size=OUTF,
                single_packet=False)
            if dbg:
                nc.sync.dma_start(
                    dbg_up[:], up_t[:].rearrange("p w f -> p (w f)"))
            out_sb = sb.tile([128, WPC * OUTF], f32)
            for w in range(WPC):
                x1Tps = ps.tile([128, 128], bf16, tag="psT", bufs=2,
                                name=f"x1T{w}")
                nc.tensor.matmul(
                    x1Tps[0:H, :], x1e[:, w * (H + 1):w * (H + 1) + H],
                    identb[:], is_transpose=True)
                x1T = sb.tile([H, 128], bf16, tag="x1T", bufs=3,
                              name=f"x1Ts{w}")
                nc.vector.tensor_copy(x1T[:], x1Tps[0:H, :])
                lgps = ps.tile([128, OUTF], f32, tag="psA", bufs=2,
                               name=f"lg{w}")
                nc.tensor.matmul(lgps[:], x1T[:], Wskb[:])
                nc.vector.tensor_tensor(
                    out_sb[:, w * OUTF:(w + 1) * OUTF], lgps[:],
                    up_t[:, w, :], ALU.add)
            nc.sync.dma_start(p_out[:], out_sb[:])

    nc.finalize()
    return nc


# --------------------------------------------------------------------------
# entry point
# --------------------------------------------------------------------------
def prepare(x, edge_index, cluster_id, W1, b1, W2, b2, W_skip, b_skip, alpha):
    meta, arr = _host_prep(np.asarray(x), np.asarray(edge_index),
                           np.asarray(cluster_id))
    BL, BH = meta["BL"], meta["BH"]

    b1 = np.asarray(b1, np.float32)
    b2 = np.asarray(b2, np.float32)
    b_skip = np.asarray(b_skip, np.float32)
    add_b1 = bool(np.any(b1 != 0))
    add_b2 = bool(np.any(b2 != 0))
    add_bsk = bool(np.any(b_skip != 0))

    nc = _build(BL, BH, add_b1, add_b2, add_bsk)

    iota = np.broadcast_to(np.arange(128, dtype=np.float32), (128, 128))
    ident = np.eye(128, dtype=np.float32)
    shared = dict(
        W1=np.ascontiguousarray(np.asarray(W1, np.float32)),
        W2=np.ascontiguousarray(np.asarray(W2, np.float32)),
        Wsk=np.ascontiguousarray(np.asarray(W_skip, np.float32)),
        b1r=b1.reshape(1, H), b2c=b2.reshape(OUTF, 1),
        bskr=b_skip.reshape(1, OUTF),
        alpha11=np.asarray(alpha, np.float32).reshape(1, 1),
        iota128=np.ascontiguousarray(iota), ident=ident,
        ahat=arr["ahat"],
    )
    in_maps = []
    for c in range(NCORES):
        m = dict(shared)
        m.update(
            xT=arr["xT"][c], degsh=arr["degsh"][c],
            gidx_lo=arr["gidx_lo"][c], gidx_hi=arr["gidx_hi"][c],
            drel_lo=arr["drel_lo"][c], drel_hi=arr["drel_hi"][c],
            cl_rel=arr["cl_rel"][c], upidx=arr["upidx"][c],
        )
        in_maps.append(m)
    return nc, in_maps, meta


def assemble(outs, meta):
    """outs: list of per-core 'out' arrays [128, WPC*OUTF]."""
    order = meta["order"]
    full = np.empty((N, OUTF), np.float32)
    for c in range(NCORES):
        o = outs[c].reshape(128, WPC, OUTF)
        o = np.ascontiguousarray(o.transpose(1, 0, 2)).reshape(SHP, OUTF)
        full[order[c * SH:(c + 1) * SH]] = o[:SH]
    return full


def kernel(x, edge_index, cluster_id, W1, b1, W2, b2, W_skip, b_skip, alpha,
           _trace=False):
    nc, in_maps, meta = prepare(x, edge_index, cluster_id, W1, b1, W2, b2,
                                W_skip, b_skip, alpha)
    from concourse.bass_utils import run_bass_kernel_spmd
    res = run_bass_kernel_spmd(nc, in_maps, list(range(NCORES)),
                               trace=_trace)
    if _trace:
        kernel.last_exec_time_ns = res.exec_time_ns
    return assemble([res.results[c]["out"] for c in range(NCORES)], meta)


# revision 20
# speedup vs baseline: 1.6448x; 1.6448x over previous
"""Trainium2 Bass kernel for LrmcSeededPoolGCN (8 NeuronCores, SPMD).

Strategy: nodes are relabeled (sorted by cluster_id) and partitioned into 8
contiguous shards of 6250 (padded 6272) nodes; edges are sharded by
destination. Each core:
  1. computes its shard of h' = dinv * (x @ W1)  (bf16, padded to 128 cols)
     and AllGathers the full table to HBM,
  2. gathers h'[src] for its (dst-sorted, window-padded) edges with
     dma_gather, reduces segments with one-hot matmuls on the TensorEngine
     (PSUM accumulation per 128-dst window); the per-edge normalization
     dinv[src]*dinv[dst] is folded into the table and the window eviction,
  3. pools x1 by cluster with the same one-hot matmul trick (ones column
     appended -> sums and counts in one shot), AllReduces [65,1024],
  4. runs the small pooled-graph GCN replicated (A_hat shipped as a dense
     0/1 matrix built host-side from the edge cluster pairs), writes
     x_p2 (+b2, +relu(alpha)*b_skip folded) k-major to HBM,
  5. gathers x_p2[cluster] per node, adds the skip matmul, writes logits.

All floating point math runs on device; the host only does integer index
preparation (permutation, edge bucketing/padding, one-hot relative indices)
and the final unshard (inverse permutation).
"""

import os
import sys

import numpy as np

if "/opt/trn_rl_repo" not in sys.path:
    sys.path.insert(0, "/opt/trn_rl_repo")

N, E, K, INF, H, OUTF = 50000, 800000, 1024, 128, 64, 64
NCORES = 8
SH = N // NCORES            # 6250 nodes per core
WPC = 49                    # 128-node windows per core
SHP = WPC * 128             # 6272 padded nodes per core
LOCUT = 32768               # int16 index split point for the gather table
TABROWS = NCORES * SHP      # 50176 rows in the all-gathered h' table
NCHUNK = 7                  # gather chunks per core (7 windows each)
WPCH = WPC // NCHUNK        # 7 windows per chunk


# --------------------------------------------------------------------------
# host-side index preparation (integer-only)
# --------------------------------------------------------------------------
def _host_prep(x, edge_index, cluster_id):
    order = np.argsort(cluster_id, kind="stable").astype(np.int64)
    inv = np.empty(N, np.int64)
    inv[order] = np.arange(N)
    clusterP = np.asarray(cluster_id, np.int64)[order]

    src = np.asarray(edge_index[0], np.int64)
    dst = np.asarray(edge_index[1], np.int64)
    deg = (np.bincount(dst, minlength=N) + 1).astype(np.float32)
    degP = deg[order]

    s_new = inv[src]
    d_new = inv[dst]
    core_of = d_new // SH
    dpos = d_new % SH
    w_of = dpos // 128
    drel_of = (dpos % 128).astype(np.float32)
    # gather-table row of the source node (shard-padded, partition-major):
    # node v in shard c at local pos l -> row c*SHP + (l%128)*WPC + l//128
    sc, sl = s_new // SH, s_new % SH
    r = sc * SHP + (sl % 128) * WPC + sl // 128
    half = (r >= LOCUT).astype(np.int64)  # 0 = lo, 1 = hi

    grp = (core_of * WPC + w_of) * 2 + half
    ngrp = NCORES * WPC * 2
    cnt = np.bincount(grp, minlength=ngrp)
    cnt_lo = cnt[0::2].reshape(NCORES, WPC)
    cnt_hi = cnt[1::2].reshape(NCORES, WPC)
    BL = max(1, int(-(-cnt_lo.max() // 128)))
    BH = max(1, int(-(-cnt_hi.max() // 128)))

    eorder = np.argsort(grp, kind="stable")
    grp_s = grp[eorder]
    starts = np.zeros(ngrp, np.int64)
    np.cumsum(cnt[:-1], out=starts[1:])
    within = np.arange(E, dtype=np.int64) - starts[grp_s]

    gidx_lo = np.zeros((NCORES, WPC * BL * 128), np.int16)
    gidx_hi = np.zeros((NCORES, WPC * BH * 128), np.int16)
    drel_lo = np.full((NCORES, WPC * BL * 128), -1.0, np.float32)
    drel_hi = np.full((NCORES, WPC * BH * 128), -1.0, np.float32)

    c_s, w_s, h_s = core_of[eorder], w_of[eorder], half[eorder]
    r_s, dr_s = r[eorder], drel_of[eorder]
    m = h_s == 0
    slot = w_s[m] * (BL * 128) + within[m]
    gidx_lo[c_s[m], slot] = r_s[m].astype(np.int16)
    drel_lo[c_s[m], slot] = dr_s[m]
    m = h_s == 1
    slot = w_s[m] * (BH * 128) + within[m]
    gidx_hi[c_s[m], slot] = (r_s[m] - LOCUT).astype(np.int16)
    drel_hi[c_s[m], slot] = dr_s[m]

    # wrap gather indices: flat slot s -> [s%16, s//16]
    def wrap16(a):
        w = a.reshape(NCORES, -1, 16).transpose(0, 2, 1)  # [NC, 16, n/16]
        return np.ascontiguousarray(np.tile(w, (1, 8, 1)))  # [NC, 128, n/16]

    # one-hot relative values: flat slot s -> [partition s%128, block s//128]
    def bywin(a):
        return np.ascontiguousarray(
            a.reshape(NCORES, -1, 128).transpose(0, 2, 1))

    # cluster one-hot relatives per node block, for the 8 global windows
    clusP = np.full((NCORES, SHP), -1, np.int64)
    for c in range(NCORES):
        clusP[c, :SH] = clusterP[c * SH:(c + 1) * SH]
    g = np.arange(8, dtype=np.int64)
    rel = clusP[:, :, None] - (g * 128)[None, None, :]  # [NC, SHP, 8]
    rel = np.where((rel >= 0) & (rel < 128), rel, -1).astype(np.float32)
    # -> [NC, 128, WPC*8]: node v=w*128+p, col w*8+g
    cl_rel = np.ascontiguousarray(
        rel.reshape(NCORES, WPC, 128, 8)
        .transpose(0, 2, 1, 3).reshape(NCORES, 128, WPC * 8))

    # x_p2 gather indices per node: row of xp2_dram == cluster id
    r2 = np.where(clusP >= 0, clusP, 0)
    upidx = wrap16(r2.astype(np.int16))  # [NC, 128, SHP//16]

    # per-core xT (feature-major, padded) and deg
    xT = np.zeros((NCORES, INF, SHP), np.float32)
    degsh = np.ones((NCORES, 128, WPC), np.float32)
    xp = np.asarray(x, np.float32)[order]
    for c in range(NCORES):
        xT[c, :, :SH] = xp[c * SH:(c + 1) * SH].T
        degsh[c] = np.pad(degP[c * SH:(c + 1) * SH],
                          (0, SHP - SH), constant_values=1.0
                          ).reshape(WPC, 128).T

    # pooled adjacency A_hat (dense 0/1 + I), rearranged [128, 8*K]
    cu = np.asarray(cluster_id, np.int64)[src]
    cv = np.asarray(cluster_id, np.int64)[dst]
    A = np.zeros((K, K), np.float32)
    A[cu, cv] = 1.0
    np.fill_diagonal(A, 1.0)
    ahat = np.ascontiguousarray(
        A.reshape(8, 128, K).transpose(1, 0, 2).reshape(128, 8 * K))

    meta = dict(order=order, BL=BL, BH=BH)
    arrays = dict(xT=xT, degsh=degsh, gidx_lo=wrap16(gidx_lo),
                  gidx_hi=wrap16(gidx_hi), drel_lo=bywin(drel_lo),
                  drel_hi=bywin(drel_hi), cl_rel=cl_rel, upidx=upidx,
                  ahat=ahat)
    return meta, arrays


# --------------------------------------------------------------------------
# device program
# --------------------------------------------------------------------------
def _build(BL, BH, add_b1, add_b2, add_bsk):
    import concourse.bacc as bacc
    import concourse.bass as bass
    import concourse.mybir as mybir
    import concourse.tile as tile

    f32 = mybir.dt.float32
    bf16 = mybir.dt.bfloat16
    i16 = mybir.dt.int16
    AF = mybir.ActivationFunctionType
    ALU = mybir.AluOpType

    nc = bacc.Bacc("TRN2", target_bir_lowering=False, debug=False,
                   num_devices=NCORES, num_swdge_queues=4)

    # ---- parameters (per-core arrays arrive via in_maps) ----
    p_xT = nc.declare_dram_parameter("xT", [INF, SHP], f32, isOutput=False)
    p_deg = nc.declare_dram_parameter("degsh", [128, WPC], f32, isOutput=False)
    p_W1 = nc.declare_dram_parameter("W1", [INF, H], f32, isOutput=False)
    p_W2 = nc.declare_dram_parameter("W2", [H, OUTF], f32, isOutput=False)
    p_Wsk = nc.declare_dram_parameter("Wsk", [H, OUTF], f32, isOutput=False)
    p_b1 = nc.declare_dram_parameter("b1r", [1, H], f32, isOutput=False)
    p_b2 = nc.declare_dram_parameter("b2c", [OUTF, 1], f32, isOutput=False)
    p_bsk = nc.declare_dram_parameter("bskr", [1, OUTF], f32, isOutput=False)
    p_alpha = nc.declare_dram_parameter("alpha11", [1, 1], f32, isOutput=False)
    MAXB = max(BL, BH, 8)
    p_iota = nc.declare_dram_parameter("iota128", [128, MAXB * 128], f32, isOutput=False)
    p_idb = nc.declare_dram_parameter("ident", [128, 128], f32, isOutput=False)
    p_glo = nc.declare_dram_parameter("gidx_lo", [128, WPC * BL * 8], i16, isOutput=False)
    p_ghi = nc.declare_dram_parameter("gidx_hi", [128, WPC * BH * 8], i16, isOutput=False)
    p_dlo = nc.declare_dram_parameter("drel_lo", [128, WPC * BL], f32, isOutput=False)
    p_dhi = nc.declare_dram_parameter("drel_hi", [128, WPC * BH], f32, isOutput=False)
    p_crel = nc.declare_dram_parameter("cl_rel", [128, WPC * 8], f32, isOutput=False)
    p_upidx = nc.declare_dram_parameter("upidx", [128, SHP // 16], i16, isOutput=False)
    p_ahat = nc.declare_dram_parameter("ahat", [128, 8 * K], f32, isOutput=False)
    p_out = nc.declare_dram_parameter("out", [128, WPC * OUTF], f32, isOutput=True)

    # ---- internal DRAM ----
    dbg = os.environ.get("BASSGNN_DEBUG") == "1"
    if dbg:
        dbg_dinvp = nc.dram_tensor("dbg_dinvp", [1, K], f32)
        dbg_xpT = nc.dram_tensor("dbg_xpT", [H, K], mybir.dt.bfloat16)
        dbg_hps = nc.dram_tensor("dbg_hps", [128, 8 * H], mybir.dt.bfloat16)
        dbg_xp2T = nc.dram_tensor("dbg_xp2T", [H, K], f32)
        dbg_up = nc.dram_tensor("dbg_up", [128, WPC * OUTF], f32)
    hsh_dram = nc.dram_tensor("hsh", [128, SHP], bf16)
    htab = nc.dram_tensor("htab", [TABROWS, 128], bf16, addr_space="Shared")
    arin = nc.dram_tensor("arin", [H + 1, K], f32)
    arout = nc.dram_tensor("arout", [H + 1, K], f32, addr_space="Shared")
    xp2_dram = nc.dram_tensor("xp2d", [K, OUTF], f32)

    cores = [list(range(NCORES))]

    with tile.TileContext(nc) as tc:
        with tc.tile_pool(name="sb", bufs=1) as sb, \
             tc.tile_pool(name="ps", bufs=1, space="PSUM") as ps:

            # ---------------- constants ----------------
            W1b = sb.tile([INF, H], bf16)
            nc.gpsimd.dma_start(W1b[:], p_W1[:])
            W2b = sb.tile([H, OUTF], bf16)
            nc.gpsimd.dma_start(W2b[:], p_W2[:])
            Wskf = sb.tile([H, OUTF], f32)
            nc.sync.dma_start(Wskf[:], p_Wsk[:])
            iota_sb = sb.tile([128, MAXB * 128], bf16)
            nc.gpsimd.dma_start(iota_sb[:], p_iota[:])
            identb = sb.tile([128, 128], bf16)
            nc.gpsimd.dma_start(identb[:], p_idb[:])
            identf = sb.tile([128, 128], f32)
            nc.sync.dma_start(identf[:], p_idb[:])
            b1row = sb.tile([1, H], f32)
            nc.sync.dma_start(b1row[:], p_b1[:])
            b2col = sb.tile([OUTF, 1], f32)
            nc.sync.dma_start(b2col[:], p_b2[:])
            bskrow = sb.tile([1, OUTF], f32)
            nc.sync.dma_start(bskrow[:], p_bsk[:])
            alpha_sb = sb.tile([1, 1], f32)
            nc.sync.dma_start(alpha_sb[:], p_alpha[:])
            deg_sb = sb.tile([128, WPC], f32)
            nc.sync.dma_start(deg_sb[:], p_deg[:])
            dlo_sb = sb.tile([128, WPC * BL], bf16)
            nc.gpsimd.dma_start(dlo_sb[:], p_dlo[:])
            dhi_sb = sb.tile([128, WPC * BH], bf16)
            nc.gpsimd.dma_start(dhi_sb[:], p_dhi[:])
            crel_sb = sb.tile([128, WPC * 8], bf16)
            nc.gpsimd.dma_start(crel_sb[:], p_crel[:])
            glo_sb = sb.tile([128, WPC * BL * 8], i16)
            nc.sync.dma_start(glo_sb[:], p_glo[:])
            ghi_sb = sb.tile([128, WPC * BH * 8], i16)
            nc.sync.dma_start(ghi_sb[:], p_ghi[:])
            upidx_sb = sb.tile([128, SHP // 16], i16)
            nc.sync.dma_start(upidx_sb[:], p_upidx[:])

            ones1x128 = sb.tile([1, 128], f32)
            nc.vector.memset(ones1x128[:], 1.0)
            onescol_b = sb.tile([128, 1], bf16)
            nc.vector.memset(onescol_b[:], 1.0)

            dinv_sb = sb.tile([128, WPC], f32)
            nc.vector.reciprocal(dinv_sb[:], deg_sb[:])
            nc.scalar.activation(dinv_sb[:], dinv_sb[:], AF.Sqrt)

            # replicated b1 [128, H]
            b1rep = sb.tile([128, H], f32)
            if add_b1:
                b1ps = ps.tile([128, H], f32, tag="psA", bufs=2)
                nc.tensor.matmul(b1ps[:], ones1x128[:], b1row[:])
                nc.vector.tensor_copy(b1rep[:], b1ps[:])

            # ---------------- phase 1: h' shard + AllGather ----------------
            xT_sb = sb.tile([INF, SHP], bf16, tag="tA", bufs=2)
            nc.gpsimd.dma_start(xT_sb[:], p_xT[:])
            hq_sb = sb.tile([128, SHP], bf16, tag="tB", bufs=2)
            nc.vector.memset(hq_sb[:], 0.0)
            hown = sb.tile([128, WPC * H], f32)
            for w in range(WPC):
                hps = ps.tile([128, H], f32, tag="psA", bufs=2, name=f"hps{w}")
                nc.tensor.matmul(hps[:], xT_sb[:, w * 128:(w + 1) * 128], W1b[:])
                nc.vector.tensor_scalar(
                    hq_sb[:, w * 128:w * 128 + H], hps[:],
                    dinv_sb[:, w:w + 1], None, ALU.mult)
                nc.vector.tensor_scalar(
                    hown[:, w * H:(w + 1) * H], hps[:],
                    dinv_sb[:, w:w + 1], None, ALU.mult)
            nc.sync.dma_start(hsh_dram[:], hq_sb[:])
            nc.gpsimd.collective_compute(
                "AllGather", mybir.AluOpType.bypass, replica_groups=cores,
                ins=[hsh_dram[:]], outs=[htab[:]])

            # ---------------- phase 2: edge gather + segment reduce --------
            x1e = sb.tile([128, WPC * (H + 1)], bf16)
            # ones column at col w*(H+1)+H for counting in the pooling matmul
            nc.vector.memset(
                x1e[:].rearrange("p (w f) -> p w f", f=H + 1)[:, :, H:H + 1], 1.0)

            nlo, nhi = WPCH * BL * 128, WPCH * BH * 128
            for chunk in range(NCHUNK):
                lo_t = sb.tile([128, WPCH * BL, 128], bf16, tag="tB", bufs=2,
                               name=f"lo{chunk}")
                nloA = (nlo // 256) * 128
                c0 = chunk * (nlo // 16)
                nc.gpsimd.dma_gather(
                    lo_t[:, :nloA // 128, :], htab[0:LOCUT, :],
                    glo_sb[:, c0:c0 + nloA // 16],
                    num_idxs=nloA, num_idxs_reg=nloA, elem_size=128,
                    single_packet=False, queue_num=0)
                nc.gpsimd.dma_gather(
                    lo_t[:, nloA // 128:, :], htab[0:LOCUT, :],
                    glo_sb[:, c0 + nloA // 16:c0 + nlo // 16],
                    num_idxs=nlo - nloA, num_idxs_reg=nlo - nloA, elem_size=128,
                    single_packet=False, queue_num=1)
                hi_t = sb.tile([128, WPCH * BH, 128], bf16, tag="tA", bufs=2,
                               name=f"hi{chunk}")
                nhiA = (nhi // 256) * 128
                c0 = chunk * (nhi // 16)
                nc.gpsimd.dma_gather(
                    hi_t[:, :nhiA // 128, :], htab[LOCUT:TABROWS, :],
                    ghi_sb[:, c0:c0 + nhiA // 16],
                    num_idxs=nhiA, num_idxs_reg=nhiA, elem_size=128,
                    single_packet=False, queue_num=2)
                nc.gpsimd.dma_gather(
                    hi_t[:, nhiA // 128:, :], htab[LOCUT:TABROWS, :],
                    ghi_sb[:, c0 + nhiA // 16:c0 + nhi // 16],
                    num_idxs=nhi - nhiA, num_idxs_reg=nhi - nhiA, elem_size=128,
                    single_packet=False, queue_num=3)

                for wl in range(WPCH):
                    w = chunk * WPCH + wl
                    # batched one-hot build for this window (lo + hi blocks)
                    oh_lo = sb.tile([128, BL, 128], bf16, tag="ohlo", bufs=3,
                                    name=f"ohlo{w}")
                    nc.vector.tensor_tensor(
                        oh_lo[:],
                        iota_sb[:, :BL * 128].rearrange("p (b j) -> p b j", j=128),
                        dlo_sb[:, w * BL:(w + 1) * BL]
                        .rearrange("p (b a) -> p b a", a=1)
                        .broadcast_to([128, BL, 128]),
                        ALU.is_equal)
                    oh_hi = sb.tile([128, BH, 128], bf16, tag="ohhi", bufs=3,
                                    name=f"ohhi{w}")
                    nc.vector.tensor_tensor(
                        oh_hi[:],
                        iota_sb[:, :BH * 128].rearrange("p (b j) -> p b j", j=128),
                        dhi_sb[:, w * BH:(w + 1) * BH]
                        .rearrange("p (b a) -> p b a", a=1)
                        .broadcast_to([128, BH, 128]),
                        ALU.is_equal)

                    eps = ps.tile([128, H], f32, tag="psA", bufs=2,
                                  name=f"eps{w}")
                    nb = BL + BH
                    for b in range(BL):
                        nc.tensor.matmul(
                            eps[:], oh_lo[:, b, :], lo_t[:, wl * BL + b, 0:H],
                            start=(b == 0), stop=(b == nb - 1))
                    for b in range(BH):
                        nc.tensor.matmul(
                            eps[:], oh_hi[:, b, :], hi_t[:, wl * BH + b, 0:H],
                            start=False, stop=(BL + b == nb - 1))

                    t1 = sb.tile([128, H], f32, tag="t1", bufs=3, name=f"t1{w}")
                    nc.vector.tensor_tensor(
                        t1[:], eps[:], hown[:, w * H:(w + 1) * H], ALU.add)
                    if add_b1:
                        nc.vector.tensor_scalar(
                            t1[:], t1[:], dinv_sb[:, w:w + 1], None, ALU.mult)
                        nc.vector.tensor_tensor(t1[:], t1[:], b1rep[:], ALU.add)
                        nc.scalar.activation(
                            x1e[:, w * (H + 1):w * (H + 1) + H], t1[:], AF.Relu)
                    else:
                        # x1 = relu(dinv * (segsum + h'own)) fused in ACT
                        nc.scalar.activation(
                            x1e[:, w * (H + 1):w * (H + 1) + H], t1[:], AF.Relu,
                            scale=dinv_sb[:, w:w + 1])

            # ---------------- phase 3: pooling + AllReduce -----------------
            poolps = [ps.tile([H + 1, 512], f32, tag=f"psP{i}", bufs=1,
                              name=f"poolps{i}") for i in range(2)]
            for w in range(WPC):
                ohc = sb.tile([128, 8, 128], bf16, tag="ohc", bufs=3,
                              name=f"ohc{w}")
                nc.vector.tensor_tensor(
                    ohc[:],
                    iota_sb[:, :8 * 128].rearrange("p (b j) -> p b j", j=128),
                    crel_sb[:, w * 8:(w + 1) * 8]
                    .rearrange("p (b a) -> p b a", a=1)
                    .broadcast_to([128, 8, 128]),
                    ALU.is_equal)
                for hf in range(2):
                    nc.tensor.matmul(
                        poolps[hf][:],
                        x1e[:, w * (H + 1):(w + 1) * (H + 1)],
                        ohc[:, 4 * hf:4 * (hf + 1), :],
                        start=(w == 0), stop=(w == WPC - 1))
            arin_sb = sb.tile([H + 1, K], f32)
            nc.vector.tensor_copy(arin_sb[:, 0:512], poolps[0][:])
            nc.vector.tensor_copy(arin_sb[:, 512:1024], poolps[1][:])
            nc.sync.dma_start(arin[:], arin_sb[:])
            nc.gpsimd.collective_compute(
                "AllReduce", mybir.AluOpType.add, replica_groups=cores,
                ins=[arin[:]], outs=[arout[:]])
            arsum = sb.tile([H + 1, K], f32)
            nc.sync.dma_start(arsum[:], arout[:])

            # ---------------- phase 4: pooled-graph GCN (replicated) -------
            ahat_sb = sb.tile([128, 8 * K], bf16)
            nc.gpsimd.dma_start(ahat_sb[:], p_ahat[:])

            # counts -> 1/max(counts,1)
            recip = sb.tile([1, K], f32)
            nc.vector.tensor_scalar(recip[:], arsum[H:H + 1, :], 1.0, None,
                                    ALU.max)
            nc.vector.reciprocal(recip[:], recip[:])

            # deg_p = column sums of A_hat; dinv_p = rsqrt
            degps = [ps.tile([1, 512], f32, tag=f"psP{i}", bufs=1,
                             name=f"degps{i}") for i in range(2)]
            for ci in range(8):
                for hf in range(2):
                    nc.tensor.matmul(
                        degps[hf][:], onescol_b[:],
                        ahat_sb[:, ci * K + hf * 512: ci * K + (hf + 1) * 512],
                        start=(ci == 0), stop=(ci == 7))
            dinvp = sb.tile([1, K], f32)
            nc.vector.reciprocal(dinvp[:, 0:512], degps[0][:])
            nc.vector.reciprocal(dinvp[:, 512:1024], degps[1][:])
            nc.scalar.activation(dinvp[:], dinvp[:], AF.Sqrt)

            # replicate dinv_p across H partitions once (SBUF, reused twice)
            dinvrep = sb.tile([H, K], f32)
            for hf in range(2):
                cs = slice(hf * 512, (hf + 1) * 512)
                dreps = ps.tile([H, 512], f32, tag="psB", bufs=2,
                                name=f"dreps{hf}")
                nc.tensor.matmul(dreps[:], ones1x128[:, 0:H], dinvp[:, cs])
                nc.vector.tensor_copy(dinvrep[:, cs], dreps[:])

            # x_pT = (sums * recip) * dinvp  (dinv_p[s] folded into x_p)
            xpT = sb.tile([H, K], bf16)
            for hf in range(2):
                cs = slice(hf * 512, (hf + 1) * 512)
                reps = ps.tile([H, 512], f32, tag="psB", bufs=2,
                               name=f"reps{hf}")
                nc.tensor.matmul(reps[:], ones1x128[:, 0:H], recip[:, cs])
                tx = sb.tile([H, 512], f32, tag="tx", bufs=2, name=f"tx{hf}")
                nc.vector.tensor_tensor(tx[:], arsum[0:H, cs], reps[:],
                                        ALU.mult)
                nc.vector.tensor_tensor(xpT[:, cs], tx[:], dinvrep[:, cs],
                                        ALU.mult)

            # h_p = (dinv_p * x_p) @ W2, s-major bf16
            hps_sb = sb.tile([128, 8 * H], bf16)
            for kb in range(8):
                hpps = ps.tile([128, H], f32, tag="psA", bufs=2,
                               name=f"hpps{kb}")
                nc.tensor.matmul(hpps[:], xpT[:, kb * 128:(kb + 1) * 128],
                                 W2b[:])
                nc.vector.tensor_copy(hps_sb[:, kb * H:(kb + 1) * H], hpps[:])

            # x_p2^T = h_p'^T @ A_hat, then * dinv_p[t] + b2
            xp2T = [ps.tile([H, 512], f32, tag=f"psP{i}", bufs=1,
                            name=f"xp2T{i}") for i in range(2)]
            for kb in range(8):
                for hf in range(2):
                    nc.tensor.matmul(
                        xp2T[hf][:],
                        hps_sb[:, kb * H:(kb + 1) * H],
                        ahat_sb[:, kb * K + hf * 512: kb * K + (hf + 1) * 512],
                        start=(kb == 0), stop=(kb == 7))
            xp2T_sb = sb.tile([H, K], f32)
            for hf in range(2):
                cs = slice(hf * 512, (hf + 1) * 512)
                nc.vector.tensor_tensor(xp2T_sb[:, cs], xp2T[hf][:],
                                        dinvrep[:, cs], ALU.mult)
                if add_b2:
                    nc.vector.tensor_scalar(xp2T_sb[:, cs], xp2T_sb[:, cs],
                                            b2col[:], None, ALU.add)

            # skip-head constants: Wsk' = relu(alpha)*Wsk ; bsk' replicated
            alr = sb.tile([1, 1], f32)
            nc.scalar.activation(alr[:], alpha_sb[:], AF.Relu)
            alr64ps = ps.tile([H, 1], f32, tag="psB", bufs=2)
            nc.tensor.matmul(alr64ps[:], ones1x128[:, 0:H], alr[:])
            alr64 = sb.tile([H, 1], f32)
            nc.vector.tensor_copy(alr64[:], alr64ps[:])
            Wskb = sb.tile([H, OUTF], bf16)
            nc.vector.tensor_scalar(Wskb[:], Wskf[:], alr64[:], None, ALU.mult)
            bsk2 = sb.tile([1, OUTF], f32)
            bskrep = sb.tile([128, OUTF], f32)
            if add_bsk:
                nc.vector.tensor_scalar(bsk2[:], bskrow[:], alr[:], None,
                                        ALU.mult)
                bskps = ps.tile([128, OUTF], f32, tag="psA", bufs=2)
                nc.tensor.matmul(bskps[:], ones1x128[:], bsk2[:])
                nc.vector.tensor_copy(bskrep[:], bskps[:])

            # x_p2 k-major [K, OUTF] f32 -> DRAM (row k at [k%128, (k//128)*64])
            xp2km = sb.tile([128, 8 * OUTF], f32)
            for kb in range(8):
                tpps = ps.tile([128, H], f32, tag="psT", bufs=2,
                               name=f"tpps{kb}")
                nc.tensor.matmul(tpps[:], xp2T_sb[:, kb * 128:(kb + 1) * 128],
                                 identf[0:H, 0:H], is_transpose=True)
                if add_bsk:
                    nc.vector.tensor_tensor(
                        xp2km[:, kb * OUTF:(kb + 1) * OUTF], tpps[:],
                        bskrep[:], ALU.add)
                else:
                    nc.vector.tensor_copy(
                        xp2km[:, kb * OUTF:(kb + 1) * OUTF], tpps[:])
            for kb in range(8):
                nc.sync.dma_start(xp2_dram[kb * 128:(kb + 1) * 128, :],
                                  xp2km[:, kb * OUTF:(kb + 1) * OUTF])

            if dbg:
                nc.sync.dma_start(dbg_dinvp[:], dinvp[:])
                nc.sync.dma_start(dbg_xpT[:], xpT[:])
                nc.sync.dma_start(dbg_hps[:], hps_sb[:])
                nc.sync.dma_start(dbg_xp2T[:], xp2T_sb[:])

            # ---------------- phase 5: per-node head -----------------------
            up_t = sb.tile([128, WPC, OUTF], f32)
            nc.gpsimd.dma_gather(
                up_t[:], xp2_dram[:], upidx_sb[:],
                num_idxs=SHP, num_idxs_reg=SHP, elem_size=OUTF,
                single_packet=False)
            if dbg:
                nc.sync.dma_start(
                    dbg_up[:], up_t[:].rearrange("p w f -> p (w f)"))
            out_sb = sb.tile([128, WPC * OUTF], f32)
            for w in range(WPC):
                x1Tps = ps.tile([128, 128], bf16, tag="psT", bufs=2,
                                name=f"x1T{w}")
                nc.tensor.matmul(
                    x1Tps[0:H, :], x1e[:, w * (H + 1):w * (H + 1) + H],
                    identb[:], is_transpose=True)
                x1T = sb.tile([H, 128], bf16, tag="x1T", bufs=3,
                              name=f"x1Ts{w}")
                nc.vector.tensor_copy(x1T[:], x1Tps[0:H, :])
                lgps = ps.tile([128, OUTF], f32, tag="psA", bufs=2,
                               name=f"lg{w}")
                nc.tensor.matmul(lgps[:], x1T[:], Wskb[:])
                nc.vector.tensor_tensor(
                    out_sb[:, w * OUTF:(w + 1) * OUTF], lgps[:],
                    up_t[:, w, :], ALU.add)
            nc.sync.dma_start(p_out[:], out_sb[:])

    nc.finalize()
    return nc


# --------------------------------------------------------------------------
# entry point
# --------------------------------------------------------------------------
def prepare(x, edge_index, cluster_id, W1, b1, W2, b2, W_skip, b_skip, alpha):
    meta, arr = _host_prep(np.asarray(x), np.asarray(edge_index),
                           np.asarray(cluster_id))
    BL, BH = meta["BL"], meta["BH"]

    b1 = np.asarray(b1, np.float32)
    b2 = np.asarray(b2, np.float32)
    b_skip = np.asarray(b_skip, np.float32)
    add_b1 = bool(np.any(b1 != 0))
    add_b2 = bool(np.any(b2 != 0))
    add_bsk = bool(np.any(b_skip != 0))

    nc = _build(BL, BH, add_b1, add_b2, add_bsk)

    MAXB = max(BL, BH, 8)
    iota = np.ascontiguousarray(
        np.broadcast_to(np.tile(np.arange(128, dtype=np.float32), MAXB),
                        (128, MAXB * 128)))
    ident = np.eye(128, dtype=np.float32)
    shared = dict(
        W1=np.ascontiguousarray(np.asarray(W1, np.float32)),
        W2=np.ascontiguousarray(np.asarray(W2, np.float32)),
        Wsk=np.ascontiguousarray(np.asarray(W_skip, np.float32)),
        b1r=b1.reshape(1, H), b2c=b2.reshape(OUTF, 1),
        bskr=b_skip.reshape(1, OUTF),
        alpha11=np.asarray(alpha, np.float32).reshape(1, 1),
        iota128=iota, ident=ident,
        ahat=arr["ahat"],
    )
    in_maps = []
    for c in range(NCORES):
        m = dict(shared)
        m.update(
            xT=arr["xT"][c], degsh=arr["degsh"][c],
            gidx_lo=arr["gidx_lo"][c], gidx_hi=arr["gidx_hi"][c],
            drel_lo=arr["drel_lo"][c], drel_hi=arr["drel_hi"][c],
            cl_rel=arr["cl_rel"][c], upidx=arr["upidx"][c],
        )
        in_maps.append(m)
    return nc, in_maps, meta


def assemble(outs, meta):
    """outs: list of per-core 'out' arrays [128, WPC*OUTF]."""
    order = meta["order"]
    full = np.empty((N, OUTF), np.float32)
    for c in range(NCORES):
        o = outs[c].reshape(128, WPC, OUTF)
        o = np.ascontiguousarray(o.transpose(1, 0, 2)).reshape(SHP, OUTF)
        full[order[c * SH:(c + 1) * SH]] = o[:SH]
    return full


def kernel(x, edge_index, cluster_id, W1, b1, W2, b2, W_skip, b_skip, alpha,
           _trace=False):
    nc, in_maps, meta = prepare(x, edge_index, cluster_id, W1, b1, W2, b2,
                                W_skip, b_skip, alpha)
    from concourse.bass_utils import run_bass_kernel_spmd
    res = run_bass_kernel_spmd(nc, in_maps, list(range(NCORES)),
                               trace=_trace)
    if _trace:
        kernel.last_exec_time_ns = res.exec_time_ns
    return assemble([res.results[c]["out"] for c in range(NCORES)], meta)
